# revision 38
# baseline (speedup 1.0000x reference)
"""Cross-attention Trainium2 kernel (8-core SPMD, no collectives).

Problem: B=4, NQ=SL=D=1024, H=16, A=64.
  q = iQ @ Wq; k,v = iK @ Wkv; scores = q k^T / sqrt(A) masked; attn = softmax;
  out = (attn v) @ Wo.  Returns (out, attn).

Sharding: core c -> batch b=c//2, head-half hh=c%2 (8 heads each).
Host pre-transposes iQ/iK/mask per batch (pure data layout); the two
partial out projections per batch are summed on host.

Per-core dataflow (proj/QK matmuls float32r; eT/v/attn-out fp16; mask fp8e5):
  qT[a,q] = Wq_s^T iQ^T      kT[a,s] = Wk_s^T iK^T      v[s,a] = iK Wv_s
  scoresT[s,q] psum = maskT-copy (identity matmul, start=True)
                    + kT_h^T qT_h  (head pairs packed on PE rows 0-63/64-127)
  eT = Exp(0.125 * psum)  fp16     (ACT; masked entries underflow to exact 0)
  oT_ext[a+1,q] += v_ext[s,a+1]^T eT    (ones column -> softmax denoms)
  attn[q,s] = PE-transpose(eT) * recip(denom)   (normalize on PSUM evict)
  out[q,D] = oT_norm^T Wo_s
Phase D is software-pipelined: iteration N's attn transposes/evicts and
oT-normalize are emitted during iteration N+1 so the reciprocal chain
(DRAM-bounce for the partition-crossing recip layout) never stalls the PE.
attn is written fp16 and upcast on host (halves the dominant DMA stream).
"""
import functools
import numpy as np

import concourse.bass as bass
import concourse.mybir as mybir
import concourse.tile as tile
from concourse import bacc
from concourse.bass_utils import run_bass_kernel_spmd

B, NQ, SL, D = 4, 1024, 1024, 1024
H, A = 16, 64
HPC = 8            # heads per core
HS = HPC * A       # 512 = per-core slice of hidden
N_CORES = 8
F32 = mybir.dt.float32
F32R = mybir.dt.float32r
BF16 = mybir.dt.bfloat16
F16 = mybir.dt.float16
FP8 = mybir.dt.float8e5
MASK_NEG = -4096.0


def _build_program():
    nc = bacc.Bacc("TRN2", target_bir_lowering=False, debug=False)

    iQT_h = nc.dram_tensor("iQT", [D, NQ], F32R, kind="ExternalInput")
    iKT_h = nc.dram_tensor("iKT", [D, SL], F32R, kind="ExternalInput")
    mnT_h = nc.dram_tensor("mnT", [SL, NQ], FP8, kind="ExternalInput")
    Wq_h = nc.dram_tensor("Wq_s", [D, HS], F32R, kind="ExternalInput")
    Wk_h = nc.dram_tensor("Wk_s", [D, HS], F32R, kind="ExternalInput")
    Wv_h = nc.dram_tensor("Wv_s", [D, HS], F32R, kind="ExternalInput")
    Wo_h = nc.dram_tensor("Wo_s", [HS, D], F32R, kind="ExternalInput")
    ident_h = nc.dram_tensor("ident", [128, 128], F32R, kind="ExternalInput")
    attn_h = nc.dram_tensor("attn_s", [HPC, NQ, SL], F16, kind="ExternalOutput")
    outp_h = nc.dram_tensor("out_p", [NQ, D], F32, kind="ExternalOutput")

    with tile.TileContext(nc) as tc:
        with (
            tc.tile_pool(name="mpool", bufs=1) as mpool,       # maskT resident
            tc.tile_pool(name="actpool", bufs=1) as actpool,   # kT,qT,v_ext,oT resident
            tc.tile_pool(name="stage", bufs=6) as stage,       # attn/out staging
            tc.tile_pool(name="small", bufs=3) as small,
            tc.tile_pool(name="const", bufs=1) as cpool,
            tc.tile_pool(name="dpool", bufs=4, space="DRAM") as dpool,
            tc.tile_pool(name="psA", bufs=7, space="PSUM") as psA,
            tc.tile_pool(name="psB", bufs=1, space="PSUM") as psB,
        ):
            # ---- constants
            ident_t = cpool.tile([128, 128], F32R)
            nc.sync.dma_start(out=ident_t, in_=ident_h.ap())
            ones_f32 = cpool.tile([128, 64], F32)
            nc.vector.memset(ones_f32, 1.0)
            ones_t = cpool.tile([128, 64], F32R)
            nc.vector.tensor_copy(ones_t, ones_f32)
            ident_bf = cpool.tile([128, 128], FP8)
            nc.vector.tensor_copy(ident_bf, ident_t)
            ident_f16 = cpool.tile([128, 128], F16)
            nc.vector.tensor_copy(ident_f16, ident_t)

            mnT_t = mpool.tile([128, 8, NQ], FP8)
            # per-ao/so/ko tiles so Tile tracks fine-grained deps
            kTs = [actpool.tile([128, SL], F32R, name=f"kT{ao}", tag=f"kT{ao}")
                   for ao in range(4)]                    # [(2h,a), s]
            vs = [actpool.tile([128, HPC, 65], F16, name=f"v{so}", tag=f"v{so}")
                  for so in range(8)]                     # [s_in, h, a+ones]
            qTs = [actpool.tile([128, NQ], F32R, name=f"qT{ao}", tag=f"qT{ao}")
                   for ao in range(4)]
            oTs = [actpool.tile([128, NQ], F32R, name=f"oT{ko}", tag=f"oT{ko}")
                   for ko in range(4)]                    # packed normalized oT

            # ---- phases B/C: projections (input pools released afterwards)
            with (
                tc.tile_pool(name="big_in", bufs=2) as big_in,
                tc.tile_pool(name="wproj", bufs=2) as wproj,
            ):
                iKT_t = big_in.tile([128, 8, SL], F32R, tag="bigin")
                Wk_t = wproj.tile([128, 8, HS], F32R, tag="w")
                Wv_t = wproj.tile([128, 8, HS], F32R, tag="w")
                iKT_r = iKT_h.ap().rearrange("(o p) s -> p o s", p=128)
                Wk_r = Wk_h.ap().rearrange("(o p) a -> p o a", p=128)
                Wv_r = Wv_h.ap().rearrange("(o p) a -> p o a", p=128)
                for ko in range(8):
                    nc.sync.dma_start(out=iKT_t[:, ko, :], in_=iKT_r[:, ko, :])
                    nc.sync.dma_start(out=Wk_t[:, ko, :], in_=Wk_r[:, ko, :])
                for ko in range(8):
                    nc.sync.dma_start(out=Wv_t[:, ko, :], in_=Wv_r[:, ko, :])

                for ao in range(4):
                    for nt in range(2):
                        ps = psA.tile([128, 512], F32, tag="mm")
                        for ko in range(8):
                            nc.tensor.matmul(
                                ps, Wk_t[:, ko, ao * 128:(ao + 1) * 128],
                                iKT_t[:, ko, nt * 512:(nt + 1) * 512],
                                start=(ko == 0), stop=(ko == 7))
                        nc.vector.tensor_copy(
                            kTs[ao][:, nt * 512:(nt + 1) * 512], ps)
                for mo in range(8):
                    ps = psA.tile([128, 512], F32, tag="mm")
                    for ko in range(8):
                        nc.tensor.matmul(
                            ps, iKT_t[:, ko, mo * 128:(mo + 1) * 128], Wv_t[:, ko, :],
                            start=(ko == 0), stop=(ko == 7))
                    nc.vector.tensor_copy(
                        vs[mo][:, :, 0:64], ps.rearrange("p (h a) -> p h a", a=64))
                    nc.vector.tensor_copy(
                        vs[mo][:, :, 64], ones_f32[:, 0:8])

                # mask loads (needed from phase D on)
                mnT_r = mnT_h.ap().rearrange("(o p) q -> p o q", p=128)
                for so in range(8):
                    nc.sync.dma_start(out=mnT_t[:, so, :], in_=mnT_r[:, so, :])

                iQT_t = big_in.tile([128, 8, NQ], F32R, tag="bigin")
                Wq_t = wproj.tile([128, 8, HS], F32R, tag="w")
                iQT_r = iQT_h.ap().rearrange("(o p) q -> p o q", p=128)
                Wq_r = Wq_h.ap().rearrange("(o p) a -> p o a", p=128)
                for ko in range(8):
                    nc.sync.dma_start(out=iQT_t[:, ko, :], in_=iQT_r[:, ko, :])
                    nc.sync.dma_start(out=Wq_t[:, ko, :], in_=Wq_r[:, ko, :])
                for ao in range(4):
                    for nt in range(2):
                        ps = psA.tile([128, 512], F32, tag="mm")
                        for ko in range(8):
                            nc.tensor.matmul(
                                ps, Wq_t[:, ko, ao * 128:(ao + 1) * 128],
                                iQT_t[:, ko, nt * 512:(nt + 1) * 512],
                                start=(ko == 0), stop=(ko == 7))
                        nc.vector.tensor_copy(
                            qTs[ao][:, nt * 512:(nt + 1) * 512], ps)

            with (
                tc.tile_pool(name="wo_pool", bufs=1) as wo_pool,
                tc.tile_pool(name="epool", bufs=3) as epool,
            ):
                Wo_t = wo_pool.tile([128, 4, D], F32R)
                nc.sync.dma_start(
                    out=Wo_t, in_=Wo_h.ap().rearrange("(o p) d -> p o d", p=128))

                # ---- phase D: head pairs (2ho, 2ho+1) x q-halves,
                # software-pipelined: iteration N's attn transposes/evicts are
                # emitted during iteration N+1 (recip chain is ready by then).
                def emit_scores(ho, qh):
                    q0 = qh * 512
                    eTs = [[epool.tile([128, 512], F16, tag=f"eT{i}_{so}",
                                       name=f"eT{i}_{so}")
                            for so in range(8)] for i in range(2)]
                    for so in range(8):
                        pss = [psA.tile([128, 512], F32, tag="mm", name="ps0"),
                               psA.tile([128, 512], F32, tag="mm", name="ps1")]
                        for ps in pss:
                            nc.tensor.matmul(
                                ps, ident_bf, mnT_t[:, so, q0:q0 + 512],
                                start=True, stop=False)
                        for i in range(2):
                            pb = 64 * i
                            nc.tensor.matmul(
                                pss[i], kTs[ho][pb:pb + 64,
                                                so * 128:(so + 1) * 128],
                                qTs[ho][pb:pb + 64, q0:q0 + 512],
                                start=False, stop=True, tile_position=(pb, 0))
                        for i in range(2):
                            nc.scalar.activation(
                                eTs[i][so], pss[i],
                                mybir.ActivationFunctionType.Exp, scale=0.125)
                    return eTs

                def emit_av(ho, qh, eTs):
                    rcs = []
                    chain = []
                    for i in range(2):
                        h = 2 * ho + i
                        eT_t = eTs[i]
                        po = psB.tile([65, 512], F32, tag="oT", name="po")
                        for so in range(8):
                            nc.tensor.matmul(
                                po, vs[so][:, h, :], eT_t[so],
                                start=(so == 0), stop=(so == 7))
                        # quick-evict po so the PSUM bank frees immediately
                        po_s = small.tile([65, 512], F32R, tag="po_s",
                                          name="po_s")
                        nc.vector.tensor_copy(po_s, po)
                        rt = small.tile([128, 512], F32R, tag="rt", name="rt")
                        with nc.allow_low_precision(reason="f32r=4B fp32"):
                            nc.vector.reciprocal(rt[64:65, :], po_s[64:65, :])
                        # recip per-partition layout via DRAM bounce
                        scr = dpool.tile([512], F32R, tag="scr", name="scr")
                        nc.sync.dma_start(out=scr, in_=rt[64:65, :])
                        rc = small.tile([128, 4], F32R, tag="rc", name="rc",
                                        bufs=4)
                        nc.sync.dma_start(
                            out=rc, in_=scr.rearrange("(j p) -> p j", p=128))
                        rcs.append(rc)
                        chain.append((po_s, rt))
                    return rcs, chain

                def emit_ot_norm(ho, qh, chain):
                    q0 = qh * 512
                    for i in range(2):
                        po_s, rt = chain[i]
                        # broadcast recip along a for oT normalize
                        bc = psA.tile([128, 512], F32, tag="mm", name="bc")
                        nc.tensor.matmul(
                            bc[0:64, :], ones_t[64:65, :], rt[64:65, :],
                            start=True, stop=True)
                        oT_ev = small.tile([64, 512], F32R, tag="oT_ev",
                                           name="oT_ev")
                        nc.vector.tensor_mul(oT_ev, bc[0:64, :], po_s[0:64, :])
                        pb = 64 * i
                        nc.sync.dma_start(
                            out=oTs[ho][pb:pb + 64, q0:q0 + 512], in_=oT_ev)

                def emit_attn_out(ho, qh, eTs, rcs):
                    q0 = qh * 512
                    for i in range(2):
                        h = 2 * ho + i
                        eT_t = eTs[i]
                        rc = rcs[i]
                        for qc in range(4):
                            at = stage.tile([128, 1024], F16, tag="attn",
                                            name="at")
                            rc_f32 = rc[:, qc:qc + 1].bitcast(F32)
                            for sh in range(2):
                                pt = psA.tile([128, 512], F16, tag="mm",
                                              name="pt")
                                for j in range(4):
                                    nc.tensor.transpose(
                                        pt[:, j * 128:(j + 1) * 128],
                                        eT_t[sh * 4 + j][:,
                                             qc * 128:(qc + 1) * 128],
                                        ident_f16)
                                nc.vector.tensor_scalar_mul(
                                    at[:, sh * 512:(sh + 1) * 512], pt, rc_f32)
                            nc.sync.dma_start(
                                out=attn_h.ap()[
                                    h, q0 + qc * 128:q0 + (qc + 1) * 128, :],
                                in_=at)

                prev = None
                for ho in range(4):
                    for qh in range(2):
                        eTs = emit_scores(ho, qh)
                        if prev is not None:
                            pho, pqh, peTs, prcs, pchain = prev
                            emit_ot_norm(pho, pqh, pchain)
                            emit_attn_out(pho, pqh, peTs, prcs)
                        rcs, chain = emit_av(ho, qh, eTs)
                        prev = (ho, qh, eTs, rcs, chain)
                pho, pqh, peTs, prcs, pchain = prev
                emit_ot_norm(pho, pqh, pchain)
                emit_attn_out(pho, pqh, peTs, prcs)

                # ---- phase E: out_p = oT^T @ Wo
                for qt in range(8):
                    ot = stage.tile([128, 1024], F32, tag="out", bufs=2)
                    for nt in range(2):
                        ps = psA.tile([128, 512], F32, tag="mm")
                        for ko in range(4):
                            nc.tensor.matmul(
                                ps, oTs[ko][:, qt * 128:(qt + 1) * 128],
                                Wo_t[:, ko, nt * 512:(nt + 1) * 512],
                                start=(ko == 0), stop=(ko == 3))
                        nc.vector.tensor_copy(
                            ot[:, nt * 512:(nt + 1) * 512], ps)
                    nc.sync.dma_start(
                        out=outp_h.ap()[qt * 128:(qt + 1) * 128, :], in_=ot)

    nc.finalize()
    return nc


@functools.lru_cache(maxsize=1)
def _get_program():
    return _build_program()


def _shard_inputs(iQ, iK, mask, Wq, Wkv, Wo):
    iQ = np.asarray(iQ, dtype=np.float32)
    iK = np.asarray(iK, dtype=np.float32)
    mask = np.asarray(mask)
    Wq = np.asarray(Wq, dtype=np.float32)
    Wkv = np.asarray(Wkv, dtype=np.float32).reshape(D, 2, H, A)
    Wo = np.asarray(Wo, dtype=np.float32)
    ident = np.eye(128, dtype=np.float32)

    iQT = [np.ascontiguousarray(iQ[b].T) for b in range(B)]
    iKT = [np.ascontiguousarray(iK[b].T) for b in range(B)]
    import ml_dtypes
    mnT = [np.ascontiguousarray(
        (mask[b].T.astype(np.float32) * MASK_NEG).astype(ml_dtypes.float8_e5m2))
        for b in range(B)]

    in_maps = []
    for c in range(N_CORES):
        b, hh = c // 2, c % 2
        h0 = hh * HPC
        in_maps.append({
            "iQT": iQT[b],
            "iKT": iKT[b],
            "mnT": mnT[b],
            "Wq_s": np.ascontiguousarray(
                Wq.reshape(D, H, A)[:, h0:h0 + HPC].reshape(D, HS)),
            "Wk_s": np.ascontiguousarray(
                Wkv[:, 0, h0:h0 + HPC].reshape(D, HS)),
            "Wv_s": np.ascontiguousarray(
                Wkv[:, 1, h0:h0 + HPC].reshape(D, HS)),
            "Wo_s": np.ascontiguousarray(Wo[h0 * A:(h0 + HPC) * A, :]),
            "ident": ident,
        })
    return in_maps


def kernel(iQ, iK, mask, Wq, Wkv, Wo):
    nc = _get_program()
    in_maps = _shard_inputs(iQ, iK, mask, Wq, Wkv, Wo)
    res = run_bass_kernel_spmd(nc, in_maps, core_ids=list(range(N_CORES)))
    out = np.zeros((B, NQ, D), dtype=np.float32)
    attn = np.empty((B, H, NQ, SL), dtype=np.float32)
    for c in range(N_CORES):
        b, hh = c // 2, c % 2
        out[b] += res.results[c]["out_p"]
        attn[b, hh * HPC:(hh + 1) * HPC] = res.results[c]["attn_s"].astype(
            np.float32)
    return out, attn


# revision 39
# speedup vs baseline: 1.0128x; 1.0128x over previous
"""Cross-attention Trainium2 kernel (8-core SPMD, no collectives).

Problem: B=4, NQ=SL=D=1024, H=16, A=64.
  q = iQ @ Wq; k,v = iK @ Wkv; scores = q k^T / sqrt(A) masked; attn = softmax;
  out = (attn v) @ Wo.  Returns (out, attn).

Sharding: core c -> batch b=c//2, head-half hh=c%2 (8 heads each).
Host pre-transposes iQ/iK/mask per batch (pure data layout); the two
partial out projections per batch are summed on host.

Per-core dataflow (proj/QK matmuls float32r; eT/v/attn-out fp16; mask fp8e5):
  qT[a,q] = Wq_s^T iQ^T      kT[a,s] = Wk_s^T iK^T      v[s,a] = iK Wv_s
  scoresT[s,q] psum = maskT-copy (identity matmul, start=True)
                    + kT_h^T qT_h  (head pairs packed on PE rows 0-63/64-127)
  eT = Exp(0.125 * psum)  fp16     (ACT; masked entries underflow to exact 0)
  oT_ext[a+1,q] += v_ext[s,a+1]^T eT    (ones column -> softmax denoms)
  attn[q,s] = PE-transpose(eT) * recip(denom)   (normalize on PSUM evict)
  out[q,D] = oT_norm^T Wo_s
Phase D is software-pipelined: iteration N's attn transposes/evicts and
oT-normalize are emitted during iteration N+1 so the reciprocal chain
(DRAM-bounce for the partition-crossing recip layout) never stalls the PE.
attn is written fp16 and upcast on host (halves the dominant DMA stream).
"""
import functools
import numpy as np

import concourse.bass as bass
import concourse.mybir as mybir
import concourse.tile as tile
from concourse import bacc
from concourse.bass_utils import run_bass_kernel_spmd

B, NQ, SL, D = 4, 1024, 1024, 1024
H, A = 16, 64
HPC = 8            # heads per core
HS = HPC * A       # 512 = per-core slice of hidden
N_CORES = 8
F32 = mybir.dt.float32
F32R = mybir.dt.float32r
BF16 = mybir.dt.bfloat16
F16 = mybir.dt.float16
FP8 = mybir.dt.float8e5
MASK_NEG = -4096.0


def _build_program():
    nc = bacc.Bacc("TRN2", target_bir_lowering=False, debug=False)

    iQT_h = nc.dram_tensor("iQT", [D, NQ], F32R, kind="ExternalInput")
    iKT_h = nc.dram_tensor("iKT", [D, SL], F32R, kind="ExternalInput")
    mnT_h = nc.dram_tensor("mnT", [SL, NQ], FP8, kind="ExternalInput")
    Wq_h = nc.dram_tensor("Wq_s", [D, HS], F32R, kind="ExternalInput")
    Wk_h = nc.dram_tensor("Wk_s", [D, HS], F32R, kind="ExternalInput")
    Wv_h = nc.dram_tensor("Wv_s", [D, HS], F32R, kind="ExternalInput")
    Wo_h = nc.dram_tensor("Wo_s", [HS, D], F32R, kind="ExternalInput")
    ident_h = nc.dram_tensor("ident", [128, 128], F32R, kind="ExternalInput")
    attn_h = nc.dram_tensor("attn_s", [HPC, NQ, SL], F16, kind="ExternalOutput")
    outp_h = nc.dram_tensor("out_p", [NQ, D], F16, kind="ExternalOutput")

    with tile.TileContext(nc) as tc:
        with (
            tc.tile_pool(name="mpool", bufs=1) as mpool,       # maskT resident
            tc.tile_pool(name="actpool", bufs=1) as actpool,   # kT,qT,v_ext,oT resident
            tc.tile_pool(name="stage", bufs=6) as stage,       # attn/out staging
            tc.tile_pool(name="small", bufs=3) as small,
            tc.tile_pool(name="const", bufs=1) as cpool,
            tc.tile_pool(name="dpool", bufs=4, space="DRAM") as dpool,
            tc.tile_pool(name="psA", bufs=7, space="PSUM") as psA,
            tc.tile_pool(name="psB", bufs=1, space="PSUM") as psB,
        ):
            # ---- constants
            ident_t = cpool.tile([128, 128], F32R)
            nc.sync.dma_start(out=ident_t, in_=ident_h.ap())
            ones_f32 = cpool.tile([128, 64], F32)
            nc.vector.memset(ones_f32, 1.0)
            ones_t = cpool.tile([128, 64], F32R)
            nc.vector.tensor_copy(ones_t, ones_f32)
            ident_bf = cpool.tile([128, 128], FP8)
            nc.vector.tensor_copy(ident_bf, ident_t)
            ident_f16 = cpool.tile([128, 128], F16)
            nc.vector.tensor_copy(ident_f16, ident_t)

            mnT_t = mpool.tile([128, 8, NQ], FP8)
            # per-ao/so/ko tiles so Tile tracks fine-grained deps
            kTs = [actpool.tile([128, SL], F32R, name=f"kT{ao}", tag=f"kT{ao}")
                   for ao in range(4)]                    # [(2h,a), s]
            vs = [actpool.tile([128, HPC, 65], F16, name=f"v{so}", tag=f"v{so}")
                  for so in range(8)]                     # [s_in, h, a+ones]
            qTs = [actpool.tile([128, NQ], F32R, name=f"qT{ao}", tag=f"qT{ao}")
                   for ao in range(4)]
            oTs = [actpool.tile([128, NQ], F32R, name=f"oT{ko}", tag=f"oT{ko}")
                   for ko in range(4)]                    # packed normalized oT

            # ---- phases B/C: projections (input pools released afterwards)
            with (
                tc.tile_pool(name="big_in", bufs=2) as big_in,
                tc.tile_pool(name="wproj", bufs=2) as wproj,
            ):
                iKT_t = big_in.tile([128, 8, SL], F32R, tag="bigin")
                Wk_t = wproj.tile([128, 8, HS], F32R, tag="w")
                Wv_t = wproj.tile([128, 8, HS], F32R, tag="w")
                iKT_r = iKT_h.ap().rearrange("(o p) s -> p o s", p=128)
                Wk_r = Wk_h.ap().rearrange("(o p) a -> p o a", p=128)
                Wv_r = Wv_h.ap().rearrange("(o p) a -> p o a", p=128)
                nc.sync.dma_start(out=Wk_t[:, 0, 0:128], in_=Wk_r[:, 0, 0:128])
                nc.sync.dma_start(out=iKT_t[:, 0, 0:512], in_=iKT_r[:, 0, 0:512])
                nc.sync.dma_start(out=Wk_t[:, 0, 128:], in_=Wk_r[:, 0, 128:])
                nc.sync.dma_start(out=iKT_t[:, 0, 512:], in_=iKT_r[:, 0, 512:])
                for ko in range(1, 8):
                    nc.sync.dma_start(out=iKT_t[:, ko, :], in_=iKT_r[:, ko, :])
                    nc.sync.dma_start(out=Wk_t[:, ko, :], in_=Wk_r[:, ko, :])
                for ko in range(8):
                    nc.sync.dma_start(out=Wv_t[:, ko, :], in_=Wv_r[:, ko, :])

                for ao in range(4):
                    for nt in range(2):
                        ps = psA.tile([128, 512], F32, tag="mm")
                        for ko in range(8):
                            nc.tensor.matmul(
                                ps, Wk_t[:, ko, ao * 128:(ao + 1) * 128],
                                iKT_t[:, ko, nt * 512:(nt + 1) * 512],
                                start=(ko == 0), stop=(ko == 7))
                        nc.vector.tensor_copy(
                            kTs[ao][:, nt * 512:(nt + 1) * 512], ps)
                for mo in range(8):
                    ps = psA.tile([128, 512], F32, tag="mm")
                    for ko in range(8):
                        nc.tensor.matmul(
                            ps, iKT_t[:, ko, mo * 128:(mo + 1) * 128], Wv_t[:, ko, :],
                            start=(ko == 0), stop=(ko == 7))
                    nc.vector.tensor_copy(
                        vs[mo][:, :, 0:64], ps.rearrange("p (h a) -> p h a", a=64))
                    nc.vector.tensor_copy(
                        vs[mo][:, :, 64], ones_f32[:, 0:8])

                # mask loads (needed from phase D on)
                mnT_r = mnT_h.ap().rearrange("(o p) q -> p o q", p=128)
                for so in range(8):
                    nc.sync.dma_start(out=mnT_t[:, so, :], in_=mnT_r[:, so, :])

                iQT_t = big_in.tile([128, 8, NQ], F32R, tag="bigin")
                Wq_t = wproj.tile([128, 8, HS], F32R, tag="w")
                iQT_r = iQT_h.ap().rearrange("(o p) q -> p o q", p=128)
                Wq_r = Wq_h.ap().rearrange("(o p) a -> p o a", p=128)
                for ko in range(8):
                    nc.sync.dma_start(out=iQT_t[:, ko, :], in_=iQT_r[:, ko, :])
                    nc.sync.dma_start(out=Wq_t[:, ko, :], in_=Wq_r[:, ko, :])
                for ao in range(4):
                    for nt in range(2):
                        ps = psA.tile([128, 512], F32, tag="mm")
                        for ko in range(8):
                            nc.tensor.matmul(
                                ps, Wq_t[:, ko, ao * 128:(ao + 1) * 128],
                                iQT_t[:, ko, nt * 512:(nt + 1) * 512],
                                start=(ko == 0), stop=(ko == 7))
                        nc.vector.tensor_copy(
                            qTs[ao][:, nt * 512:(nt + 1) * 512], ps)

            with (
                tc.tile_pool(name="wo_pool", bufs=1) as wo_pool,
                tc.tile_pool(name="epool", bufs=3) as epool,
            ):
                Wo_t = wo_pool.tile([128, 4, D], F32R)
                nc.sync.dma_start(
                    out=Wo_t, in_=Wo_h.ap().rearrange("(o p) d -> p o d", p=128))

                # ---- phase D: head pairs (2ho, 2ho+1) x q-halves,
                # software-pipelined: iteration N's attn transposes/evicts are
                # emitted during iteration N+1 (recip chain is ready by then).
                def emit_scores(ho, qh):
                    q0 = qh * 512
                    eTs = [[epool.tile([128, 512], F16, tag=f"eT{i}_{so}",
                                       name=f"eT{i}_{so}")
                            for so in range(8)] for i in range(2)]
                    for so in range(8):
                        pss = [psA.tile([128, 512], F32, tag="mm", name="ps0"),
                               psA.tile([128, 512], F32, tag="mm", name="ps1")]
                        for ps in pss:
                            nc.tensor.matmul(
                                ps, ident_bf, mnT_t[:, so, q0:q0 + 512],
                                start=True, stop=False)
                        for i in range(2):
                            pb = 64 * i
                            nc.tensor.matmul(
                                pss[i], kTs[ho][pb:pb + 64,
                                                so * 128:(so + 1) * 128],
                                qTs[ho][pb:pb + 64, q0:q0 + 512],
                                start=False, stop=True, tile_position=(pb, 0))
                        for i in range(2):
                            nc.scalar.activation(
                                eTs[i][so], pss[i],
                                mybir.ActivationFunctionType.Exp, scale=0.125)
                    return eTs

                def emit_av(ho, qh, eTs):
                    rcs = []
                    chain = []
                    for i in range(2):
                        h = 2 * ho + i
                        eT_t = eTs[i]
                        po = psB.tile([65, 512], F32, tag="oT", name="po")
                        for so in range(8):
                            nc.tensor.matmul(
                                po, vs[so][:, h, :], eT_t[so],
                                start=(so == 0), stop=(so == 7))
                        # quick-evict po so the PSUM bank frees immediately
                        po_s = small.tile([65, 512], F32R, tag="po_s",
                                          name="po_s")
                        nc.vector.tensor_copy(po_s, po)
                        rt = small.tile([128, 512], F32R, tag="rt", name="rt")
                        with nc.allow_low_precision(reason="f32r=4B fp32"):
                            nc.vector.reciprocal(rt[64:65, :], po_s[64:65, :])
                        # recip per-partition layout via DRAM bounce
                        scr = dpool.tile([512], F32R, tag="scr", name="scr")
                        nc.sync.dma_start(out=scr, in_=rt[64:65, :])
                        rc = small.tile([128, 4], F32R, tag="rc", name="rc",
                                        bufs=4)
                        nc.sync.dma_start(
                            out=rc, in_=scr.rearrange("(j p) -> p j", p=128))
                        rcs.append(rc)
                        chain.append((po_s, rt))
                    return rcs, chain

                def emit_ot_norm(ho, qh, chain):
                    q0 = qh * 512
                    for i in range(2):
                        po_s, rt = chain[i]
                        # broadcast recip along a for oT normalize
                        bc = psA.tile([128, 512], F32, tag="mm", name="bc")
                        nc.tensor.matmul(
                            bc[0:64, :], ones_t[64:65, :], rt[64:65, :],
                            start=True, stop=True)
                        oT_ev = small.tile([64, 512], F32R, tag="oT_ev",
                                           name="oT_ev")
                        nc.vector.tensor_mul(oT_ev, bc[0:64, :], po_s[0:64, :])
                        pb = 64 * i
                        nc.sync.dma_start(
                            out=oTs[ho][pb:pb + 64, q0:q0 + 512], in_=oT_ev)

                def emit_attn_out(ho, qh, eTs, rcs):
                    q0 = qh * 512
                    for i in range(2):
                        h = 2 * ho + i
                        eT_t = eTs[i]
                        rc = rcs[i]
                        for qc in range(4):
                            at = stage.tile([128, 1024], F16, tag="attn",
                                            name="at")
                            rc_f32 = rc[:, qc:qc + 1].bitcast(F32)
                            for sh in range(2):
                                pt = psA.tile([128, 512], F16, tag="mm",
                                              name="pt")
                                for j in range(4):
                                    nc.tensor.transpose(
                                        pt[:, j * 128:(j + 1) * 128],
                                        eT_t[sh * 4 + j][:,
                                             qc * 128:(qc + 1) * 128],
                                        ident_f16)
                                nc.vector.tensor_scalar_mul(
                                    at[:, sh * 512:(sh + 1) * 512], pt, rc_f32)
                            nc.sync.dma_start(
                                out=attn_h.ap()[
                                    h, q0 + qc * 128:q0 + (qc + 1) * 128, :],
                                in_=at)

                prev = None
                for ho in range(4):
                    for qh in range(2):
                        eTs = emit_scores(ho, qh)
                        if prev is not None:
                            pho, pqh, peTs, prcs, pchain = prev
                            emit_ot_norm(pho, pqh, pchain)
                            emit_attn_out(pho, pqh, peTs, prcs)
                        rcs, chain = emit_av(ho, qh, eTs)
                        prev = (ho, qh, eTs, rcs, chain)
                pho, pqh, peTs, prcs, pchain = prev
                emit_ot_norm(pho, pqh, pchain)
                emit_attn_out(pho, pqh, peTs, prcs)

                # ---- phase E: out_p = oT^T @ Wo
                for qt in range(8):
                    ot = stage.tile([128, 1024], F16, tag="out", bufs=2)
                    for nt in range(2):
                        ps = psA.tile([128, 512], F32, tag="mm")
                        for ko in range(4):
                            nc.tensor.matmul(
                                ps, oTs[ko][:, qt * 128:(qt + 1) * 128],
                                Wo_t[:, ko, nt * 512:(nt + 1) * 512],
                                start=(ko == 0), stop=(ko == 3))
                        nc.vector.tensor_copy(
                            ot[:, nt * 512:(nt + 1) * 512], ps)
                    nc.sync.dma_start(
                        out=outp_h.ap()[qt * 128:(qt + 1) * 128, :], in_=ot)

    nc.finalize()
    return nc


@functools.lru_cache(maxsize=1)
def _get_program():
    return _build_program()


def _shard_inputs(iQ, iK, mask, Wq, Wkv, Wo):
    iQ = np.asarray(iQ, dtype=np.float32)
    iK = np.asarray(iK, dtype=np.float32)
    mask = np.asarray(mask)
    Wq = np.asarray(Wq, dtype=np.float32)
    Wkv = np.asarray(Wkv, dtype=np.float32).reshape(D, 2, H, A)
    Wo = np.asarray(Wo, dtype=np.float32)
    ident = np.eye(128, dtype=np.float32)

    iQT = [np.ascontiguousarray(iQ[b].T) for b in range(B)]
    iKT = [np.ascontiguousarray(iK[b].T) for b in range(B)]
    import ml_dtypes
    mnT = [np.ascontiguousarray(
        (mask[b].T.astype(np.float32) * MASK_NEG).astype(ml_dtypes.float8_e5m2))
        for b in range(B)]

    in_maps = []
    for c in range(N_CORES):
        b, hh = c // 2, c % 2
        h0 = hh * HPC
        in_maps.append({
            "iQT": iQT[b],
            "iKT": iKT[b],
            "mnT": mnT[b],
            "Wq_s": np.ascontiguousarray(
                Wq.reshape(D, H, A)[:, h0:h0 + HPC].reshape(D, HS)),
            "Wk_s": np.ascontiguousarray(
                Wkv[:, 0, h0:h0 + HPC].reshape(D, HS)),
            "Wv_s": np.ascontiguousarray(
                Wkv[:, 1, h0:h0 + HPC].reshape(D, HS)),
            "Wo_s": np.ascontiguousarray(Wo[h0 * A:(h0 + HPC) * A, :]),
            "ident": ident,
        })
    return in_maps


def kernel(iQ, iK, mask, Wq, Wkv, Wo):
    nc = _get_program()
    in_maps = _shard_inputs(iQ, iK, mask, Wq, Wkv, Wo)
    res = run_bass_kernel_spmd(nc, in_maps, core_ids=list(range(N_CORES)))
    out = np.zeros((B, NQ, D), dtype=np.float32)
    attn = np.empty((B, H, NQ, SL), dtype=np.float32)
    for c in range(N_CORES):
        b, hh = c // 2, c % 2
        out[b] += res.results[c]["out_p"].astype(np.float32)
        attn[b, hh * HPC:(hh + 1) * HPC] = res.results[c]["attn_s"].astype(
            np.float32)
    return out, attn


# revision 41
# speedup vs baseline: 1.0462x; 1.0331x over previous
"""Cross-attention Trainium2 kernel (8-core SPMD, no collectives).

Problem: B=4, NQ=SL=D=1024, H=16, A=64.
  q = iQ @ Wq; k,v = iK @ Wkv; scores = q k^T / sqrt(A) masked; attn = softmax;
  out = (attn v) @ Wo.  Returns (out, attn).

Sharding: core c -> batch b=c//2, head-half hh=c%2 (8 heads each).
Host pre-transposes iQ/iK/mask per batch (pure data layout); the two
partial out projections per batch are summed on host.

Per-core dataflow (proj/QK matmuls float32r; eT/v/attn-out fp16; mask fp8e5):
  qT[a,q] = Wq_s^T iQ^T      kT[a,s] = Wk_s^T iK^T      v[s,a] = iK Wv_s
  scoresT[s,q] psum = maskT-copy (identity matmul, start=True)
                    + kT_h^T qT_h  (head pairs packed on PE rows 0-63/64-127)
  eT = Exp(0.125 * psum)  fp16     (ACT; masked entries underflow to exact 0)
  oT_ext[a+1,q] += v_ext[s,a+1]^T eT    (ones column -> softmax denoms)
  attn[q,s] = PE-transpose(eT) * recip(denom)   (normalize on PSUM evict)
  out[q,D] = oT_norm^T Wo_s
Phase D is software-pipelined: iteration N's attn transposes/evicts and
oT-normalize are emitted during iteration N+1 so the reciprocal chain
(DRAM-bounce for the partition-crossing recip layout) never stalls the PE.
attn is written fp16 and upcast on host (halves the dominant DMA stream).
"""
import functools
import numpy as np

import concourse.bass as bass
import concourse.mybir as mybir
import concourse.tile as tile
from concourse import bacc
from concourse.bass_utils import run_bass_kernel_spmd

B, NQ, SL, D = 4, 1024, 1024, 1024
H, A = 16, 64
HPC = 8            # heads per core
HS = HPC * A       # 512 = per-core slice of hidden
N_CORES = 8
F32 = mybir.dt.float32
F32R = mybir.dt.float32r
BF16 = mybir.dt.bfloat16
F16 = mybir.dt.float16
FP8 = mybir.dt.float8e5
MASK_NEG = -4096.0


def _build_program():
    nc = bacc.Bacc("TRN2", target_bir_lowering=False, debug=False)

    iQT_h = nc.dram_tensor("iQT", [D, NQ], F32R, kind="ExternalInput")
    iKT_h = nc.dram_tensor("iKT", [D, SL], F32R, kind="ExternalInput")
    mnT_h = nc.dram_tensor("mnT", [SL, NQ], FP8, kind="ExternalInput")
    Wq_h = nc.dram_tensor("Wq_s", [D, HS], F32R, kind="ExternalInput")
    Wk_h = nc.dram_tensor("Wk_s", [D, HS], F32R, kind="ExternalInput")
    Wv_h = nc.dram_tensor("Wv_s", [D, HS], F32R, kind="ExternalInput")
    Wo_h = nc.dram_tensor("Wo_s", [HS, D], F32R, kind="ExternalInput")
    ident_h = nc.dram_tensor("ident", [128, 128], F32R, kind="ExternalInput")
    attn_h = nc.dram_tensor("attn_s", [HPC, NQ, SL], F16, kind="ExternalOutput")
    outp_h = nc.dram_tensor("out_p", [NQ, D], F16, kind="ExternalOutput")

    with tile.TileContext(nc) as tc:
        with (
            tc.tile_pool(name="mpool", bufs=1) as mpool,       # maskT resident
            tc.tile_pool(name="actpool", bufs=1) as actpool,   # kT,qT,v_ext,oT resident
            tc.tile_pool(name="stage", bufs=6) as stage,       # attn/out staging
            tc.tile_pool(name="small", bufs=3) as small,
            tc.tile_pool(name="const", bufs=1) as cpool,
            tc.tile_pool(name="dpool", bufs=4, space="DRAM") as dpool,
            tc.tile_pool(name="psA", bufs=7, space="PSUM") as psA,
            tc.tile_pool(name="psB", bufs=1, space="PSUM") as psB,
        ):
            # ---- constants
            ident_t = cpool.tile([128, 128], F32R)
            nc.sync.dma_start(out=ident_t, in_=ident_h.ap())
            ones_f32 = cpool.tile([128, 64], F32)
            nc.vector.memset(ones_f32, 1.0)
            ones_t = cpool.tile([128, 64], F32R)
            nc.vector.tensor_copy(ones_t, ones_f32)
            ident_bf = cpool.tile([128, 128], FP8)
            nc.vector.tensor_copy(ident_bf, ident_t)
            ident_f16 = cpool.tile([128, 128], F16)
            nc.vector.tensor_copy(ident_f16, ident_t)

            mnT_t = mpool.tile([128, 8, NQ], FP8)
            # per-ao/so/ko tiles so Tile tracks fine-grained deps
            kTs = [actpool.tile([128, SL], F32R, name=f"kT{ao}", tag=f"kT{ao}")
                   for ao in range(4)]                    # [(2h,a), s]
            vs = [actpool.tile([128, HPC, 65], F16, name=f"v{so}", tag=f"v{so}")
                  for so in range(8)]                     # [s_in, h, a+ones]
            qTs = [actpool.tile([128, NQ], F32R, name=f"qT{ao}", tag=f"qT{ao}")
                   for ao in range(4)]
            oTs = [[actpool.tile([128, 512], F32R, name=f"oT{ko}_{qh}",
                                 tag=f"oT{ko}_{qh}") for qh in range(2)]
                   for ko in range(4)]                    # packed normalized oT

            # ---- phases B/C: projections (input pools released afterwards)
            with (
                tc.tile_pool(name="big_in", bufs=2) as big_in,
                tc.tile_pool(name="wproj", bufs=2) as wproj,
            ):
                iKT_t = big_in.tile([128, 8, SL], F32R, tag="bigin")
                Wk_t = wproj.tile([128, 8, HS], F32R, tag="w")
                Wv_t = wproj.tile([128, 8, HS], F32R, tag="w")
                iKT_r = iKT_h.ap().rearrange("(o p) s -> p o s", p=128)
                Wk_r = Wk_h.ap().rearrange("(o p) a -> p o a", p=128)
                Wv_r = Wv_h.ap().rearrange("(o p) a -> p o a", p=128)
                nc.sync.dma_start(out=Wk_t[:, 0, 0:128], in_=Wk_r[:, 0, 0:128])
                nc.sync.dma_start(out=iKT_t[:, 0, 0:512], in_=iKT_r[:, 0, 0:512])
                nc.sync.dma_start(out=Wk_t[:, 0, 128:], in_=Wk_r[:, 0, 128:])
                nc.sync.dma_start(out=iKT_t[:, 0, 512:], in_=iKT_r[:, 0, 512:])
                for ko in range(1, 8):
                    nc.sync.dma_start(out=iKT_t[:, ko, :], in_=iKT_r[:, ko, :])
                    nc.sync.dma_start(out=Wk_t[:, ko, :], in_=Wk_r[:, ko, :])
                for ko in range(8):
                    nc.sync.dma_start(out=Wv_t[:, ko, :], in_=Wv_r[:, ko, :])

                for ao in range(4):
                    for nt in range(2):
                        ps = psA.tile([128, 512], F32, tag="mm")
                        for ko in range(8):
                            nc.tensor.matmul(
                                ps, Wk_t[:, ko, ao * 128:(ao + 1) * 128],
                                iKT_t[:, ko, nt * 512:(nt + 1) * 512],
                                start=(ko == 0), stop=(ko == 7))
                        nc.vector.tensor_copy(
                            kTs[ao][:, nt * 512:(nt + 1) * 512], ps)
                for mo in range(8):
                    ps = psA.tile([128, 512], F32, tag="mm")
                    for ko in range(8):
                        nc.tensor.matmul(
                            ps, iKT_t[:, ko, mo * 128:(mo + 1) * 128], Wv_t[:, ko, :],
                            start=(ko == 0), stop=(ko == 7))
                    nc.vector.tensor_copy(
                        vs[mo][:, :, 0:64], ps.rearrange("p (h a) -> p h a", a=64))
                    nc.vector.tensor_copy(
                        vs[mo][:, :, 64], ones_f32[:, 0:8])

                # mask loads (needed from phase D on)
                mnT_r = mnT_h.ap().rearrange("(o p) q -> p o q", p=128)
                for so in range(8):
                    nc.sync.dma_start(out=mnT_t[:, so, :], in_=mnT_r[:, so, :])

                iQT_t = big_in.tile([128, 8, NQ], F32R, tag="bigin")
                Wq_t = wproj.tile([128, 8, HS], F32R, tag="w")
                iQT_r = iQT_h.ap().rearrange("(o p) q -> p o q", p=128)
                Wq_r = Wq_h.ap().rearrange("(o p) a -> p o a", p=128)
                for ko in range(8):
                    nc.sync.dma_start(out=iQT_t[:, ko, :], in_=iQT_r[:, ko, :])
                    nc.sync.dma_start(out=Wq_t[:, ko, :], in_=Wq_r[:, ko, :])
                for ao in range(4):
                    for nt in range(2):
                        ps = psA.tile([128, 512], F32, tag="mm")
                        for ko in range(8):
                            nc.tensor.matmul(
                                ps, Wq_t[:, ko, ao * 128:(ao + 1) * 128],
                                iQT_t[:, ko, nt * 512:(nt + 1) * 512],
                                start=(ko == 0), stop=(ko == 7))
                        nc.vector.tensor_copy(
                            qTs[ao][:, nt * 512:(nt + 1) * 512], ps)

            with (
                tc.tile_pool(name="wo_pool", bufs=1) as wo_pool,
                tc.tile_pool(name="epool", bufs=3) as epool,
            ):
                Wo_t = wo_pool.tile([128, 4, D], F32R)
                nc.sync.dma_start(
                    out=Wo_t, in_=Wo_h.ap().rearrange("(o p) d -> p o d", p=128))

                # ---- phase D: head pairs (2ho, 2ho+1) x q-halves,
                # software-pipelined: iteration N's attn transposes/evicts are
                # emitted during iteration N+1 (recip chain is ready by then).
                def emit_scores(ho, qh):
                    q0 = qh * 512
                    eTs = [[epool.tile([128, 512], F16, tag=f"eT{i}_{so}",
                                       name=f"eT{i}_{so}")
                            for so in range(8)] for i in range(2)]
                    for so in range(8):
                        pss = [psA.tile([128, 512], F32, tag="mm", name="ps0"),
                               psA.tile([128, 512], F32, tag="mm", name="ps1")]
                        for ps in pss:
                            nc.tensor.matmul(
                                ps, ident_bf, mnT_t[:, so, q0:q0 + 512],
                                start=True, stop=False)
                        for i in range(2):
                            pb = 64 * i
                            nc.tensor.matmul(
                                pss[i], kTs[ho][pb:pb + 64,
                                                so * 128:(so + 1) * 128],
                                qTs[ho][pb:pb + 64, q0:q0 + 512],
                                start=False, stop=True, tile_position=(pb, 0))
                        for i in range(2):
                            nc.scalar.activation(
                                eTs[i][so], pss[i],
                                mybir.ActivationFunctionType.Exp, scale=0.125)
                    return eTs

                def emit_av(ho, qh, eTs):
                    rcs = []
                    chain = []
                    for i in range(2):
                        h = 2 * ho + i
                        eT_t = eTs[i]
                        po = psB.tile([65, 512], F32, tag="oT", name="po")
                        for so in range(8):
                            nc.tensor.matmul(
                                po, vs[so][:, h, :], eT_t[so],
                                start=(so == 0), stop=(so == 7))
                        # quick-evict po so the PSUM bank frees immediately
                        po_s = small.tile([65, 512], F32R, tag="po_s",
                                          name="po_s")
                        nc.vector.tensor_copy(po_s, po)
                        rt = small.tile([128, 512], F32R, tag="rt", name="rt")
                        with nc.allow_low_precision(reason="f32r=4B fp32"):
                            nc.vector.reciprocal(rt[64:65, :], po_s[64:65, :])
                        # recip per-partition layout via DRAM bounce
                        scr = dpool.tile([512], F32R, tag="scr", name="scr")
                        nc.sync.dma_start(out=scr, in_=rt[64:65, :])
                        rc = small.tile([128, 4], F32R, tag="rc", name="rc",
                                        bufs=4)
                        nc.sync.dma_start(
                            out=rc, in_=scr.rearrange("(j p) -> p j", p=128))
                        rcs.append(rc)
                        chain.append((po_s, rt))
                    return rcs, chain

                def emit_ot_norm(ho, qh, chain):
                    q0 = qh * 512
                    for i in range(2):
                        po_s, rt = chain[i]
                        # broadcast recip along a for oT normalize
                        bc = psA.tile([128, 512], F32, tag="mm", name="bc")
                        nc.tensor.matmul(
                            bc[0:64, :], ones_t[64:65, :], rt[64:65, :],
                            start=True, stop=True)
                        oT_ev = small.tile([64, 512], F32R, tag="oT_ev",
                                           name="oT_ev")
                        nc.vector.tensor_mul(oT_ev, bc[0:64, :], po_s[0:64, :])
                        pb = 64 * i
                        nc.sync.dma_start(
                            out=oTs[ho][qh][pb:pb + 64, :], in_=oT_ev)

                def emit_attn_out(ho, qh, eTs, rcs):
                    q0 = qh * 512
                    for i in range(2):
                        h = 2 * ho + i
                        eT_t = eTs[i]
                        rc = rcs[i]
                        for qc in range(4):
                            at = stage.tile([128, 1024], F16, tag="attn",
                                            name="at")
                            rc_f32 = rc[:, qc:qc + 1].bitcast(F32)
                            for sh in range(2):
                                pt = psA.tile([128, 512], F16, tag="mm",
                                              name="pt")
                                for j in range(4):
                                    nc.tensor.transpose(
                                        pt[:, j * 128:(j + 1) * 128],
                                        eT_t[sh * 4 + j][:,
                                             qc * 128:(qc + 1) * 128],
                                        ident_f16)
                                nc.vector.tensor_scalar_mul(
                                    at[:, sh * 512:(sh + 1) * 512], pt, rc_f32)
                            nc.sync.dma_start(
                                out=attn_h.ap()[
                                    h, q0 + qc * 128:q0 + (qc + 1) * 128, :],
                                in_=at)

                def emit_out_proj(qts):
                    for qt in qts:
                        ot = stage.tile([128, 1024], F16, tag="out", bufs=2,
                                        name="ot")
                        for nt in range(2):
                            ps = psA.tile([128, 512], F32, tag="mm", name="ps")
                            for ko in range(4):
                                nc.tensor.matmul(
                                    ps, oTs[ko][qt // 4][:, (qt % 4) * 128:
                                                         (qt % 4 + 1) * 128],
                                    Wo_t[:, ko, nt * 512:(nt + 1) * 512],
                                    start=(ko == 0), stop=(ko == 3))
                            nc.vector.tensor_copy(
                                ot[:, nt * 512:(nt + 1) * 512], ps)
                        nc.sync.dma_start(
                            out=outp_h.ap()[qt * 128:(qt + 1) * 128, :], in_=ot)

                prev = None
                for ho in range(4):
                    for qh in range(2):
                        eTs = emit_scores(ho, qh)
                        if prev is not None:
                            pho, pqh, peTs, prcs, pchain = prev
                            emit_ot_norm(pho, pqh, pchain)
                            emit_attn_out(pho, pqh, peTs, prcs)
                        rcs, chain = emit_av(ho, qh, eTs)
                        prev = (ho, qh, eTs, rcs, chain)
                pho, pqh, peTs, prcs, pchain = prev
                emit_out_proj(range(0, 4))
                emit_ot_norm(pho, pqh, pchain)
                emit_attn_out(pho, pqh, peTs, prcs)

                # ---- phase E second half (first half interleaved above)
                emit_out_proj(range(4, 8))

    nc.finalize()
    return nc


@functools.lru_cache(maxsize=1)
def _get_program():
    return _build_program()


def _shard_inputs(iQ, iK, mask, Wq, Wkv, Wo):
    iQ = np.asarray(iQ, dtype=np.float32)
    iK = np.asarray(iK, dtype=np.float32)
    mask = np.asarray(mask)
    Wq = np.asarray(Wq, dtype=np.float32)
    Wkv = np.asarray(Wkv, dtype=np.float32).reshape(D, 2, H, A)
    Wo = np.asarray(Wo, dtype=np.float32)
    ident = np.eye(128, dtype=np.float32)

    iQT = [np.ascontiguousarray(iQ[b].T) for b in range(B)]
    iKT = [np.ascontiguousarray(iK[b].T) for b in range(B)]
    import ml_dtypes
    mnT = [np.ascontiguousarray(
        (mask[b].T.astype(np.float32) * MASK_NEG).astype(ml_dtypes.float8_e5m2))
        for b in range(B)]

    in_maps = []
    for c in range(N_CORES):
        b, hh = c // 2, c % 2
        h0 = hh * HPC
        in_maps.append({
            "iQT": iQT[b],
            "iKT": iKT[b],
            "mnT": mnT[b],
            "Wq_s": np.ascontiguousarray(
                Wq.reshape(D, H, A)[:, h0:h0 + HPC].reshape(D, HS)),
            "Wk_s": np.ascontiguousarray(
                Wkv[:, 0, h0:h0 + HPC].reshape(D, HS)),
            "Wv_s": np.ascontiguousarray(
                Wkv[:, 1, h0:h0 + HPC].reshape(D, HS)),
            "Wo_s": np.ascontiguousarray(Wo[h0 * A:(h0 + HPC) * A, :]),
            "ident": ident,
        })
    return in_maps


def kernel(iQ, iK, mask, Wq, Wkv, Wo):
    nc = _get_program()
    in_maps = _shard_inputs(iQ, iK, mask, Wq, Wkv, Wo)
    res = run_bass_kernel_spmd(nc, in_maps, core_ids=list(range(N_CORES)))
    out = np.zeros((B, NQ, D), dtype=np.float32)
    attn = np.empty((B, H, NQ, SL), dtype=np.float32)
    for c in range(N_CORES):
        b, hh = c // 2, c % 2
        out[b] += res.results[c]["out_p"].astype(np.float32)
        attn[b, hh * HPC:(hh + 1) * HPC] = res.results[c]["attn_s"].astype(
            np.float32)
    return out, attn
